# revision 40
# baseline (speedup 1.0000x reference)
"""Trainium2 Bass kernel for nn_DiffusionModule (self-similarity diffusion).

Math (per batch b, with src = feature_src[b].reshape(C, N)):
    P   = src^T @ src                      # [N, N], sim = P / sqrt(C)
    mu_n = mean_m P[m, n]  (P symmetric)
    aff[n, m] = exp(-((P[n,m] - mu_n) / (16*sqrt(2)))^2)   # sigma=1, C=256
    D = aff / rowsum(aff)
    out = 0.5 * (src @ D^T) + 0.5 * dst

Key identity: P[m,n] - mu_n = sum_c (src[c,m] - sbar[c]) * src[c,n] where
sbar[c] = mean_m src[c,m].  So centering the m-side operand of the first
matmul performs the row-mean subtraction for free (no rank-1 updates).

Sharding: 8 cores = 4 batches x 2 column-halves. SPMD.

Per-core layout (everything in "transposed" [m partitions, n free] space):
  - mm1: simT psum [128m, 512n] = ONE fp8 DoubleRow matmul (K=256 via k-pair)
    using csrc (centered src, fp8) x srcr (raw src cols, fp8)
  - Square -> y2 fp32 (split between ScalarE and VectorE to balance engines)
  - Exp on ScalarE (scale=-1/512 folds the (16*sqrt2))^2 scaling) -> aff bf16
  - mm2: aff chunks as lhsT (K=m), srcT bf16 (with ones column -> row-sums
    land in column 256) as rhs -> out2 psum [128n, 257]
  - normalize rows on DVE, bf16 PE transpose back to [c, n], blend 0.5*dst,
    DMA out.
  - srcT built by DMA-xbar transpose of a bf16 copy of src (no PE/fp32 work).
"""

import os
import threading

import numpy as np

_KERNEL_CACHE = {}
_LOCK = threading.Lock()

B, C, H, W = 4, 256, 64, 64
N = 4096  # H*W
HALF = N // 2  # columns per core
NBLK = 512  # n-block width
N_NBLK = HALF // NBLK  # 4
MT = N // 128  # 32 m-tiles
KC = C // 128  # 2 contraction chunks
SCL2 = 1.0 / 512.0  # ((P-mu)/(16*sqrt2))^2 == SCL2 * (P-mu)^2
ALPHA = 0.5
EPS = 1e-12


def _build():
    """Build + compile the SPMD Bass program once. Returns nc."""
    from contextlib import ExitStack

    import concourse.bass as bass
    import concourse.tile as tile
    from concourse import bacc, mybir
    from concourse.masks import make_identity

    fp32 = mybir.dt.float32
    bf16 = mybir.dt.bfloat16
    fp8 = mybir.dt.float8e4

    nc = bacc.Bacc(
        "TRN2", target_bir_lowering=False, debug=False, num_devices=8
    )

    src_d = nc.dram_tensor("src", [C, N], fp32, kind="ExternalInput").ap()
    dst_d = nc.dram_tensor("dst", [C, HALF], fp32, kind="ExternalInput").ap()
    out_d = nc.dram_tensor("out", [C, HALF], fp32, kind="ExternalOutput").ap()

    reps = int(os.environ.get("KERNEL_REPS", "1"))
    # of every 16 2-tile square groups, this many go to DVE (rest ScalarE)
    dve_sq = int(os.environ.get("KERNEL_DVE_SQ", "10"))


    with tile.TileContext(nc) as tc, ExitStack() as ctx:
        singles = ctx.enter_context(tc.tile_pool(name="singles", bufs=1))
        pspool = ctx.enter_context(tc.tile_pool(name="ps", bufs=2, space="PSUM"))
        opool = ctx.enter_context(tc.tile_pool(name="o", bufs=4, space="PSUM"))
        y2pool = ctx.enter_context(tc.tile_pool(name="y2", bufs=2))
        zpool = ctx.enter_context(tc.tile_pool(name="z", bufs=3))
        affpool = ctx.enter_context(tc.tile_pool(name="aff", bufs=2))
        outpool = ctx.enter_context(tc.tile_pool(name="outsb", bufs=4))
        smallp = ctx.enter_context(tc.tile_pool(name="small", bufs=8))

        for _rep in range(reps):
            # ---------------- stage 0: loads + prep ----------------
            sb_src = singles.tile([128, KC, N], fp32)
            for k in range(KC):
                nc.sync.dma_start(
                    sb_src[:, k, :],
                    src_d.rearrange("(k p) n -> k p n", p=128)[k],
                )
            sb_dst = singles.tile([128, KC, HALF], fp32)
            nc.sync.dma_start(sb_dst, dst_d.rearrange("(k p) n -> p k n", p=128))

            identity16 = singles.tile([128, 128], bf16)
            make_identity(nc, identity16)

            # bf16 copy of src (for srcT transpose) fused with the row-sum
            # for the mean: ScalarE Copy + accum_out in one pass per k-chunk.
            src16 = singles.tile([128, KC, N], bf16)
            negmean = singles.tile([128, KC], fp32)
            csrc = singles.tile([128, KC, N], fp8)
            for k in range(KC):
                nc.scalar.activation(
                    src16[:, k, :],
                    sb_src[:, k, :],
                    mybir.ActivationFunctionType.Copy,
                    accum_out=negmean[:, k : k + 1],
                )
                nc.vector.tensor_scalar_mul(
                    negmean[:, k : k + 1], negmean[:, k : k + 1], -1.0 / float(N)
                )
                # centered src (fp8) for mm1 lhsT
                nc.vector.tensor_scalar_add(
                    csrc[:, k, :], sb_src[:, k, :], negmean[:, k : k + 1]
                )
            # srcr: nb0's slice now; the tail is emitted inside nb0 so it
            # doesn't head-block the DVE queue at main-loop start
            srcr = singles.tile([128, KC, HALF], fp8)
            nc.vector.tensor_copy(srcr[:, :, 0:NBLK], sb_src[:, :, 0:NBLK])

            # srcT [m, c] bf16 with ones column at c=256 (row-sum trick),
            # built by DMA xbar transpose of the bf16 copy of src.
            # The xbar transpose needs a CONTIGUOUS per-partition dst on HW
            # (strided dst produces wrong output), so transpose into tmpT
            # and fan out into the strided srcT layout on DVE.
            # row padded to C+2 so per-mt row stride stays 4B-aligned; the
            # full-tile memset(1.0) supplies the ones column (col 256), the
            # transposed copies overwrite cols 0..255.
            sb_srcT = singles.tile([128, MT, C + 2], bf16)
            nc.gpsimd.memset(sb_srcT, 1.0)
            tmpT = singles.tile([128, KC, MT, 128], bf16)
            for k in range(KC):
                nc.sync.dma_start_transpose(
                    tmpT[:, k],
                    src16[:, k, :],
                )
                nc.vector.tensor_copy(
                    sb_srcT[:, :, k * 128 : (k + 1) * 128], tmpT[:, k]
                )

            # PE warmup: ~4us of transpose spins gated on late-prep data
            # (tmpT k=0) so they run right before the mm1 burst and flip
            # HAM to K=8/8 as the burst begins.
            warm_ps = opool.tile([128, 512], bf16, tag="o", name="warm")
            for _w in range(20):
                nc.tensor.transpose(
                    warm_ps[:, 0:128], tmpT[:, 0, 0, :], identity16
                )

            # ---------------- main loop over n-blocks ----------------
            pending_out = None  # thunk for previous n-block's out stage

            def emit_out_stage(po, n0):
                # Normalize on ScalarE, transpose back to [c, n] via DMA
                # xbar. Per-q emission so each chain starts as early as
                # possible. Returns a thunk with the blend+store, deferred
                # so the DVE STT never head-blocks the square-path copies.
                otT = outpool.tile([128, KC, NBLK], bf16, tag="otT", name="otT")
                for q in range(4):
                    sq = smallp.tile([128, 1], fp32, name="sq")
                    nc.vector.tensor_scalar(
                        sq,
                        po[q][:, C : C + 1],
                        EPS,
                        1.0 / ALPHA,
                        op0=mybir.AluOpType.max,
                        op1=mybir.AluOpType.mult,
                    )
                    nc.vector.reciprocal(sq, sq)
                    ot = outpool.tile([128, C], bf16, tag="outT", name="outT")
                    nc.scalar.mul(ot, po[q][:, 0:C], sq)
                    for cb in range(KC):
                        nc.sync.dma_start_transpose(
                            otT[:, cb, q * 128 : (q + 1) * 128],
                            ot[:, cb * 128 : (cb + 1) * 128],
                        )

                def blend_and_store():
                    for cb in range(KC):
                        ob = outpool.tile([128, NBLK], fp32, tag="ob", name="ob")
                        nc.vector.scalar_tensor_tensor(
                            ob,
                            sb_dst[:, cb, n0 : n0 + NBLK],
                            ALPHA,
                            otT[:, cb, :],
                            op0=mybir.AluOpType.mult,
                            op1=mybir.AluOpType.add,
                        )
                        nc.sync.dma_start(
                            out_d[cb * 128 : (cb + 1) * 128, n0 : n0 + NBLK], ob
                        )

                return blend_and_store

            pending_blend = None  # deferred blend+store of prev out-stage
            for nb in range(N_NBLK):
                n0 = nb * NBLK
                po = [
                    opool.tile([128, 512], fp32, tag="o", name=f"po{q}")
                    for q in range(4)
                ]
                ps_sim = None
                y2 = None
                mm2_q = []  # deferred 2nd-matmul chunks: (afft, g8, j)

                def emit_mm2_chunk():
                    afft_, g8_, j_ = mm2_q.pop(0)
                    mtg = g8_ * 8 + j_
                    for q in range(4):
                        nc.tensor.matmul(
                            po[q][:, 0 : C + 1],
                            afft_[:, j_, q * 128 : (q + 1) * 128],
                            sb_srcT[:, mtg, 0 : C + 1],
                            start=(mtg == 0),
                            stop=(mtg == MT - 1),
                        )

                for mt in range(MT):
                    gi = mt % 2
                    if gi == 0:
                        ps_sim = pspool.tile(
                            [128, 2, NBLK], fp32, tag="g", name="ps_sim"
                        )
                    # mm1: single fp8 DoubleRow matmul, K=256 (2 k-tiles)
                    nc.tensor.matmul(
                        ps_sim[:, gi, :],
                        csrc[:, :, mt * 128 : (mt + 1) * 128],
                        srcr[:, :, n0 : n0 + NBLK],
                        start=True,
                        stop=True,
                        perf_mode=mybir.MatmulPerfMode.DoubleRow,
                    )
                    if mm2_q:
                        emit_mm2_chunk()
                    if mt % 8 == 0:
                        y2 = y2pool.tile([128, 8, NBLK], bf16, name="y2")
                    if gi == 1:
                        # square the 2-tile group -> y2 half (no scaling:
                        # y2 = (P-mu)^2; Exp applies -1/512)
                        base = ((mt % 8) // 2) * 2
                        g2 = (nb * MT + mt) // 2 % 16
                        # ScalarE takes the MIDDLE band of groups: at the
                        # n-block boundary the slot-holding squares are then
                        # on the (fast-releasing) DVE path, not queued
                        # behind a 3.9us EXP in ScalarE's strict FIFO.
                        if not (5 <= g2 < 16 - (dve_sq - 5)):
                            # DVE may read PSUM on only one input: copy to
                            # SBUF bf16 (1 PSUM read), then square at 2x.
                            zt = zpool.tile([128, 2, NBLK], bf16, name="zt")
                            nc.vector.tensor_copy(zt, ps_sim)
                            nc.vector.tensor_mul(
                                y2[:, base : base + 2, :], zt, zt
                            )
                        else:
                            nc.scalar.activation(
                                y2[:, base : base + 2, :],
                                ps_sim,
                                mybir.ActivationFunctionType.Square,
                                scale=1.0,
                            )
                    if mt % 8 == 7:
                        g8 = mt // 8
                        afft = affpool.tile([128, 8, NBLK], bf16, name="afft")
                        nc.scalar.activation(
                            afft,
                            y2,
                            mybir.ActivationFunctionType.Exp,
                            scale=-SCL2,
                        )
                        for j in range(8):
                            mm2_q.append((afft, g8, j))
                    if mt == 3 and pending_out is not None:
                        pending_blend = pending_out()
                        pending_out = None
                    if mt == 8 and nb == 0:
                        # srcr tail cast, safely mid-stream on the DVE
                        nc.vector.tensor_copy(
                            srcr[:, :, NBLK:], sb_src[:, :, NBLK:HALF]
                        )
                    if mt == 16 and pending_blend is not None:
                        pending_blend()
                        pending_blend = None
                while mm2_q:
                    emit_mm2_chunk()
                pending_out = (lambda po=po, n0=n0: emit_out_stage(po, n0))
            pending_blend = pending_out()
            pending_out = None
            pending_blend()
            pending_blend = None

    nc.compile()
    return nc


def _get_compiled():
    with _LOCK:
        key = (
            os.environ.get("KERNEL_REPS", "1"),
            os.environ.get("KERNEL_DVE_SQ", "10"),
        )
        if key not in _KERNEL_CACHE:
            _KERNEL_CACHE[key] = _build()
        return _KERNEL_CACHE[key]


def _make_in_maps(feature_src, feature_dst):
    src = np.ascontiguousarray(
        np.asarray(feature_src, dtype=np.float32).reshape(B, C, N)
    )
    dst = np.ascontiguousarray(
        np.asarray(feature_dst, dtype=np.float32).reshape(B, C, N)
    )
    in_maps = []
    for core in range(8):
        b, h = core // 2, core % 2
        sl = slice(h * HALF, (h + 1) * HALF)
        in_maps.append(
            {
                # roll so this core's column-half sits at columns 0:HALF;
                # the m-axis permutation cancels in both matmul contractions
                "src": np.ascontiguousarray(np.roll(src[b], -h * HALF, axis=1)),
                "dst": np.ascontiguousarray(dst[b][:, sl]),
            }
        )
    return in_maps


def _assemble(results):
    out = np.empty((B, C, N), dtype=np.float32)
    for core in range(8):
        b, h = core // 2, core % 2
        out[b][:, h * HALF : (h + 1) * HALF] = results[core]["out"]
    return out.reshape(B, C, H, W)


def run(feature_src, feature_dst, trace=False):
    """Run on 8 NeuronCores; returns (output [B,C,H,W], exec_time_ns|None)."""
    from concourse import bass_utils

    nc = _get_compiled()
    in_maps = _make_in_maps(feature_src, feature_dst)
    res = bass_utils.run_bass_kernel_spmd(
        nc, in_maps, core_ids=list(range(8)), trace=trace
    )
    return _assemble(res.results), res.exec_time_ns


def kernel(feature_src, feature_dst):
    out, _ = run(feature_src, feature_dst, trace=False)
    return out


# revision 41
# speedup vs baseline: 1.1975x; 1.1975x over previous
"""Trainium2 Bass kernel for nn_DiffusionModule (self-similarity diffusion).

Math (per batch b, with src = feature_src[b].reshape(C, N)):
    P   = src^T @ src                      # [N, N], sim = P / sqrt(C)
    mu_n = mean_m P[m, n]  (P symmetric)
    aff[n, m] = exp(-((P[n,m] - mu_n) / (16*sqrt(2)))^2)   # sigma=1, C=256
    D = aff / rowsum(aff)
    out = 0.5 * (src @ D^T) + 0.5 * dst

Key identity: P[m,n] - mu_n = sum_c (src[c,m] - sbar[c]) * src[c,n] where
sbar[c] = mean_m src[c,m].  So centering the m-side operand of the first
matmul performs the row-mean subtraction for free (no rank-1 updates).

Sharding: 8 cores = 4 batches x 2 column-halves. SPMD.

Per-core layout (everything in "transposed" [m partitions, n free] space):
  - mm1: simT psum [128m, 512n] = ONE fp8 DoubleRow matmul (K=256 via k-pair)
    using csrc (centered src, fp8) x srcr (raw src cols, fp8)
  - Square -> y2 fp32 (split between ScalarE and VectorE to balance engines)
  - Exp on ScalarE (scale=-1/512 folds the (16*sqrt2))^2 scaling) -> aff bf16
  - mm2: aff chunks as lhsT (K=m), srcT bf16 (with ones column -> row-sums
    land in column 256) as rhs -> out2 psum [128n, 257]
  - normalize rows on DVE, bf16 PE transpose back to [c, n], blend 0.5*dst,
    DMA out.
  - srcT built by DMA-xbar transpose of a bf16 copy of src (no PE/fp32 work).
"""

import os
import threading

import numpy as np

_KERNEL_CACHE = {}
_LOCK = threading.Lock()

B, C, H, W = 4, 256, 64, 64
N = 4096  # H*W
HALF = N // 2  # columns per core
NBLK = 512  # n-block width
N_NBLK = HALF // NBLK  # 4
MT = N // 128  # 32 m-tiles
KC = C // 128  # 2 contraction chunks
SCL2 = 1.0 / 512.0  # ((P-mu)/(16*sqrt2))^2 == SCL2 * (P-mu)^2
ALPHA = 0.5
EPS = 1e-12


def _build():
    """Build + compile the SPMD Bass program once. Returns nc."""
    from contextlib import ExitStack

    import concourse.bass as bass
    import concourse.tile as tile
    from concourse import bacc, mybir
    from concourse.masks import make_identity

    fp32 = mybir.dt.float32
    bf16 = mybir.dt.bfloat16
    fp8 = mybir.dt.float8e4

    nc = bacc.Bacc(
        "TRN2", target_bir_lowering=False, debug=False, num_devices=8
    )

    src_d = nc.dram_tensor("src", [C, N], fp32, kind="ExternalInput").ap()
    dst_d = nc.dram_tensor("dst", [C, HALF], fp32, kind="ExternalInput").ap()
    out_d = nc.dram_tensor("out", [C, HALF], fp32, kind="ExternalOutput").ap()

    reps = int(os.environ.get("KERNEL_REPS", "1"))
    # of every 16 2-tile square groups, this many go to DVE (rest ScalarE)
    dve_sq = int(os.environ.get("KERNEL_DVE_SQ", "10"))


    with tile.TileContext(nc) as tc, ExitStack() as ctx:
        singles = ctx.enter_context(tc.tile_pool(name="singles", bufs=1))
        pspool = ctx.enter_context(tc.tile_pool(name="ps", bufs=2, space="PSUM"))
        opool = ctx.enter_context(tc.tile_pool(name="o", bufs=4, space="PSUM"))
        y2pool = ctx.enter_context(tc.tile_pool(name="y2", bufs=2))
        zpool = ctx.enter_context(tc.tile_pool(name="z", bufs=3))
        affpool = ctx.enter_context(tc.tile_pool(name="aff", bufs=2))
        outpool = ctx.enter_context(tc.tile_pool(name="outsb", bufs=4))
        smallp = ctx.enter_context(tc.tile_pool(name="small", bufs=8))

        for _rep in range(reps):
            # ---------------- stage 0: loads + prep ----------------
            sb_src = singles.tile([128, KC, N], fp32)
            for k in range(KC):
                nc.sync.dma_start(
                    sb_src[:, k, :],
                    src_d.rearrange("(k p) n -> k p n", p=128)[k],
                )
            sb_dst = singles.tile([128, KC, HALF], fp32)
            nc.sync.dma_start(sb_dst, dst_d.rearrange("(k p) n -> p k n", p=128))

            identity16 = singles.tile([128, 128], bf16)
            make_identity(nc, identity16)

            # bf16 copy of src (for srcT transpose) fused with the row-sum
            # for the mean: ScalarE Copy + accum_out in one pass per k-chunk.
            src16 = singles.tile([128, KC, N], bf16)
            negmean = singles.tile([128, KC], fp32)
            csrc = singles.tile([128, KC, N], fp8)
            for k in range(KC):
                nc.scalar.activation(
                    src16[:, k, :],
                    sb_src[:, k, :],
                    mybir.ActivationFunctionType.Copy,
                    accum_out=negmean[:, k : k + 1],
                )
                nc.vector.tensor_scalar_mul(
                    negmean[:, k : k + 1], negmean[:, k : k + 1], -1.0 / float(N)
                )
                # centered src (fp8) for mm1 lhsT
                nc.vector.tensor_scalar_add(
                    csrc[:, k, :], sb_src[:, k, :], negmean[:, k : k + 1]
                )
            # srcr: nb0's slice now; the tail is emitted inside nb0 so it
            # doesn't head-block the DVE queue at main-loop start
            srcr = singles.tile([128, KC, HALF], fp8)
            nc.vector.tensor_copy(srcr[:, :, 0:NBLK], sb_src[:, :, 0:NBLK])

            # srcT [m, c] bf16 with ones column at c=256 (row-sum trick),
            # built by DMA xbar transpose of the bf16 copy of src.
            # The xbar transpose needs a CONTIGUOUS per-partition dst on HW
            # (strided dst produces wrong output), so transpose into tmpT
            # and fan out into the strided srcT layout on DVE.
            # row padded to C+2 so per-mt row stride stays 4B-aligned; the
            # full-tile memset(1.0) supplies the ones column (col 256), the
            # transposed copies overwrite cols 0..255.
            sb_srcT = singles.tile([128, MT, C + 2], bf16)
            nc.gpsimd.memset(sb_srcT, 1.0)
            tmpT = singles.tile([128, KC, MT, 128], bf16)
            for k in range(KC):
                nc.sync.dma_start_transpose(
                    tmpT[:, k],
                    src16[:, k, :],
                )
                nc.vector.tensor_copy(
                    sb_srcT[:, :, k * 128 : (k + 1) * 128], tmpT[:, k]
                )

            # PE warmup: ~4us of transpose spins gated on late-prep data
            # (tmpT k=0) so they run right before the mm1 burst and flip
            # HAM to K=8/8 as the burst begins.
            warm_ps = opool.tile([128, 512], bf16, tag="o", name="warm")
            for _w in range(20):
                nc.tensor.transpose(
                    warm_ps[:, 0:128], tmpT[:, 0, 0, :], identity16
                )

            # ---------------- main loop over n-blocks ----------------
            pending_out = None  # thunk for previous n-block's out stage

            def emit_out_stage(po, n0):
                # Normalize on ScalarE, transpose back to [c, n] via DMA
                # xbar. Per-q emission so each chain starts as early as
                # possible. Returns a thunk with the blend+store, deferred
                # so the DVE STT never head-blocks the square-path copies.
                otT = outpool.tile([128, KC, NBLK], bf16, tag="otT", name="otT")
                for q in range(4):
                    sq = smallp.tile([128, 1], fp32, name="sq")
                    nc.vector.tensor_scalar(
                        sq,
                        po[q][:, C : C + 1],
                        EPS,
                        1.0 / ALPHA,
                        op0=mybir.AluOpType.max,
                        op1=mybir.AluOpType.mult,
                    )
                    nc.vector.reciprocal(sq, sq)
                    ot = outpool.tile([128, C], bf16, tag="outT", name="outT")
                    nc.scalar.mul(ot, po[q][:, 0:C], sq)
                    for cb in range(KC):
                        nc.sync.dma_start_transpose(
                            otT[:, cb, q * 128 : (q + 1) * 128],
                            ot[:, cb * 128 : (cb + 1) * 128],
                        )

                def blend_and_store():
                    for cb in range(KC):
                        ob = outpool.tile([128, NBLK], fp32, tag="ob", name="ob")
                        nc.vector.scalar_tensor_tensor(
                            ob,
                            sb_dst[:, cb, n0 : n0 + NBLK],
                            ALPHA,
                            otT[:, cb, :],
                            op0=mybir.AluOpType.mult,
                            op1=mybir.AluOpType.add,
                        )
                        nc.sync.dma_start(
                            out_d[cb * 128 : (cb + 1) * 128, n0 : n0 + NBLK], ob
                        )

                return blend_and_store

            pending_blend = None  # deferred blend+store of prev out-stage
            for nb in range(N_NBLK):
                n0 = nb * NBLK
                po = [
                    opool.tile([128, 512], fp32, tag="o", name=f"po{q}")
                    for q in range(4)
                ]
                ps_sim = None
                y2 = None
                mm2_q = []  # deferred 2nd-matmul chunks: (afft, g8, j)

                def emit_mm2_chunk():
                    afft_, g8_, j_ = mm2_q.pop(0)
                    mtg = g8_ * 8 + j_
                    for q in range(4):
                        nc.tensor.matmul(
                            po[q][:, 0 : C + 1],
                            afft_[:, j_, q * 128 : (q + 1) * 128],
                            sb_srcT[:, mtg, 0 : C + 1],
                            start=(mtg == 0),
                            stop=(mtg == MT - 1),
                        )

                for mt in range(MT):
                    gi = mt % 2
                    if gi == 0:
                        ps_sim = pspool.tile(
                            [128, 2, NBLK], fp32, tag="g", name="ps_sim"
                        )
                    # mm1: single fp8 DoubleRow matmul, K=256 (2 k-tiles)
                    nc.tensor.matmul(
                        ps_sim[:, gi, :],
                        csrc[:, :, mt * 128 : (mt + 1) * 128],
                        srcr[:, :, n0 : n0 + NBLK],
                        start=True,
                        stop=True,
                        perf_mode=mybir.MatmulPerfMode.DoubleRow,
                    )
                    if mm2_q:
                        emit_mm2_chunk()
                    if mt % 8 == 0:
                        y2 = y2pool.tile([128, 8, NBLK], bf16, name="y2")
                    if gi == 1:
                        # square the 2-tile group -> y2 half (no scaling:
                        # y2 = (P-mu)^2; Exp applies -1/512)
                        base = ((mt % 8) // 2) * 2
                        g2 = (nb * MT + mt) // 2 % 16
                        if g2 < dve_sq:
                            # DVE may read PSUM on only one input: copy to
                            # SBUF bf16 (1 PSUM read), then square at 2x.
                            zt = zpool.tile([128, 2, NBLK], bf16, name="zt")
                            nc.vector.tensor_copy(zt, ps_sim)
                            nc.vector.tensor_mul(
                                y2[:, base : base + 2, :], zt, zt
                            )
                        else:
                            nc.scalar.activation(
                                y2[:, base : base + 2, :],
                                ps_sim,
                                mybir.ActivationFunctionType.Square,
                                scale=1.0,
                            )
                    if mt % 8 == 7:
                        g8 = mt // 8
                        afft = affpool.tile([128, 8, NBLK], bf16, name="afft")
                        nc.scalar.activation(
                            afft,
                            y2,
                            mybir.ActivationFunctionType.Exp,
                            scale=-SCL2,
                        )
                        for j in range(8):
                            mm2_q.append((afft, g8, j))
                    if mt == 3 and pending_out is not None:
                        pending_blend = pending_out()
                        pending_out = None
                    if mt == 8 and nb == 0:
                        # srcr tail cast, safely mid-stream on the DVE
                        nc.vector.tensor_copy(
                            srcr[:, :, NBLK:], sb_src[:, :, NBLK:HALF]
                        )
                    if mt == 16 and pending_blend is not None:
                        pending_blend()
                        pending_blend = None
                while mm2_q:
                    emit_mm2_chunk()
                pending_out = (lambda po=po, n0=n0: emit_out_stage(po, n0))
            pending_blend = pending_out()
            pending_out = None
            pending_blend()
            pending_blend = None

    nc.compile()
    return nc


def _get_compiled():
    with _LOCK:
        key = (
            os.environ.get("KERNEL_REPS", "1"),
            os.environ.get("KERNEL_DVE_SQ", "10"),
        )
        if key not in _KERNEL_CACHE:
            _KERNEL_CACHE[key] = _build()
        return _KERNEL_CACHE[key]


def _make_in_maps(feature_src, feature_dst):
    src = np.ascontiguousarray(
        np.asarray(feature_src, dtype=np.float32).reshape(B, C, N)
    )
    dst = np.ascontiguousarray(
        np.asarray(feature_dst, dtype=np.float32).reshape(B, C, N)
    )
    in_maps = []
    for core in range(8):
        b, h = core // 2, core % 2
        sl = slice(h * HALF, (h + 1) * HALF)
        in_maps.append(
            {
                # roll so this core's column-half sits at columns 0:HALF;
                # the m-axis permutation cancels in both matmul contractions
                "src": np.ascontiguousarray(np.roll(src[b], -h * HALF, axis=1)),
                "dst": np.ascontiguousarray(dst[b][:, sl]),
            }
        )
    return in_maps


def _assemble(results):
    out = np.empty((B, C, N), dtype=np.float32)
    for core in range(8):
        b, h = core // 2, core % 2
        out[b][:, h * HALF : (h + 1) * HALF] = results[core]["out"]
    return out.reshape(B, C, H, W)


def run(feature_src, feature_dst, trace=False):
    """Run on 8 NeuronCores; returns (output [B,C,H,W], exec_time_ns|None)."""
    from concourse import bass_utils

    nc = _get_compiled()
    in_maps = _make_in_maps(feature_src, feature_dst)
    res = bass_utils.run_bass_kernel_spmd(
        nc, in_maps, core_ids=list(range(8)), trace=trace
    )
    return _assemble(res.results), res.exec_time_ns


def kernel(feature_src, feature_dst):
    out, _ = run(feature_src, feature_dst, trace=False)
    return out


# revision 43
# speedup vs baseline: 1.2880x; 1.0756x over previous
"""Trainium2 Bass kernel for nn_DiffusionModule (self-similarity diffusion).

Math (per batch b, with src = feature_src[b].reshape(C, N)):
    P   = src^T @ src                      # [N, N], sim = P / sqrt(C)
    mu_n = mean_m P[m, n]  (P symmetric)
    aff[n, m] = exp(-((P[n,m] - mu_n) / (16*sqrt(2)))^2)   # sigma=1, C=256
    D = aff / rowsum(aff)
    out = 0.5 * (src @ D^T) + 0.5 * dst

Key identity: P[m,n] - mu_n = sum_c (src[c,m] - sbar[c]) * src[c,n] where
sbar[c] = mean_m src[c,m].  So centering the m-side operand of the first
matmul performs the row-mean subtraction for free (no rank-1 updates).

Sharding: 8 cores = 4 batches x 2 column-halves. SPMD.

Per-core layout (everything in "transposed" [m partitions, n free] space):
  - mm1: simT psum [128m, 512n] = ONE fp8 DoubleRow matmul (K=256 via k-pair)
    using csrc (centered src, fp8) x srcr (raw src cols, fp8)
  - Square -> y2 fp32 (split between ScalarE and VectorE to balance engines)
  - Exp on ScalarE (scale=-1/512 folds the (16*sqrt2))^2 scaling) -> aff bf16
  - mm2: aff chunks as lhsT (K=m), srcT bf16 (with ones column -> row-sums
    land in column 256) as rhs -> out2 psum [128n, 257]
  - normalize rows on DVE, bf16 PE transpose back to [c, n], blend 0.5*dst,
    DMA out.
  - srcT built by DMA-xbar transpose of a bf16 copy of src (no PE/fp32 work).
"""

import os
import threading

import numpy as np

_KERNEL_CACHE = {}
_LOCK = threading.Lock()

B, C, H, W = 4, 256, 64, 64
N = 4096  # H*W
HALF = N // 2  # columns per core
NBLK = 512  # n-block width
N_NBLK = HALF // NBLK  # 4
MT = N // 128  # 32 m-tiles
KC = C // 128  # 2 contraction chunks
SCL2 = 1.0 / 512.0  # ((P-mu)/(16*sqrt2))^2 == SCL2 * (P-mu)^2
ALPHA = 0.5
EPS = 1e-12


def _build():
    """Build + compile the SPMD Bass program once. Returns nc."""
    from contextlib import ExitStack

    import concourse.bass as bass
    import concourse.tile as tile
    from concourse import bacc, mybir
    from concourse.masks import make_identity

    fp32 = mybir.dt.float32
    bf16 = mybir.dt.bfloat16
    fp8 = mybir.dt.float8e4

    nc = bacc.Bacc(
        "TRN2", target_bir_lowering=False, debug=False, num_devices=8
    )

    src_d = nc.dram_tensor("src", [C, N], fp32, kind="ExternalInput").ap()
    dst_d = nc.dram_tensor("dst", [C, HALF], fp32, kind="ExternalInput").ap()
    out_d = nc.dram_tensor("out", [C, HALF], fp32, kind="ExternalOutput").ap()

    reps = int(os.environ.get("KERNEL_REPS", "1"))
    # of every 16 2-tile square groups, this many go to DVE (rest ScalarE)
    dve_sq = int(os.environ.get("KERNEL_DVE_SQ", "11"))


    with tile.TileContext(nc) as tc, ExitStack() as ctx:
        singles = ctx.enter_context(tc.tile_pool(name="singles", bufs=1))
        pspool = ctx.enter_context(tc.tile_pool(name="ps", bufs=2, space="PSUM"))
        opool = ctx.enter_context(tc.tile_pool(name="o", bufs=4, space="PSUM"))
        y2pool = ctx.enter_context(tc.tile_pool(name="y2", bufs=2))
        zpool = ctx.enter_context(tc.tile_pool(name="z", bufs=3))
        affpool = ctx.enter_context(tc.tile_pool(name="aff", bufs=2))
        outpool = ctx.enter_context(tc.tile_pool(name="outsb", bufs=4))
        smallp = ctx.enter_context(tc.tile_pool(name="small", bufs=8))

        for _rep in range(reps):
            # ---------------- stage 0: loads + prep ----------------
            sb_src = singles.tile([128, KC, N], fp32)
            for k in range(KC):
                nc.sync.dma_start(
                    sb_src[:, k, :],
                    src_d.rearrange("(k p) n -> k p n", p=128)[k],
                )
            sb_dst = singles.tile([128, KC, HALF], fp32)
            nc.sync.dma_start(sb_dst, dst_d.rearrange("(k p) n -> p k n", p=128))

            identity16 = singles.tile([128, 128], bf16)
            make_identity(nc, identity16)

            # bf16 copy of src (for srcT transpose) fused with the row-sum
            # for the mean: ScalarE Copy + accum_out in one pass per k-chunk.
            src16 = singles.tile([128, KC, N], bf16)
            negmean = singles.tile([128, KC], fp32)
            csrc = singles.tile([128, KC, N], fp8)
            for k in range(KC):
                nc.scalar.activation(
                    src16[:, k, :],
                    sb_src[:, k, :],
                    mybir.ActivationFunctionType.Copy,
                    accum_out=negmean[:, k : k + 1],
                )
                nc.vector.tensor_scalar_mul(
                    negmean[:, k : k + 1], negmean[:, k : k + 1], -1.0 / float(N)
                )
                # centered src (fp8) for mm1 lhsT
                nc.vector.tensor_scalar_add(
                    csrc[:, k, :], sb_src[:, k, :], negmean[:, k : k + 1]
                )
            # srcr: nb0's slice now; the tail is emitted inside nb0 so it
            # doesn't head-block the DVE queue at main-loop start
            srcr = singles.tile([128, KC, HALF], fp8)
            nc.vector.tensor_copy(srcr[:, :, 0:NBLK], sb_src[:, :, 0:NBLK])

            # srcT [m, c] bf16 with ones column at c=256 (row-sum trick),
            # built by DMA xbar transpose of the bf16 copy of src.
            # The xbar transpose needs a CONTIGUOUS per-partition dst on HW
            # (strided dst produces wrong output), so transpose into tmpT
            # and fan out into the strided srcT layout on DVE.
            # row padded to C+2 so per-mt row stride stays 4B-aligned; the
            # full-tile memset(1.0) supplies the ones column (col 256), the
            # transposed copies overwrite cols 0..255.
            sb_srcT = singles.tile([128, MT, C + 2], bf16)
            nc.gpsimd.memset(sb_srcT, 1.0)
            tmpT = singles.tile([128, KC, MT, 128], bf16)
            for k in range(KC):
                nc.sync.dma_start_transpose(
                    tmpT[:, k],
                    src16[:, k, :],
                )
                nc.vector.tensor_copy(
                    sb_srcT[:, :, k * 128 : (k + 1) * 128], tmpT[:, k]
                )

            # PE warmup: ~4us of transpose spins gated on late-prep data
            # (tmpT k=0) so they run right before the mm1 burst and flip
            # HAM to K=8/8 as the burst begins.
            warm_ps = opool.tile([128, 512], bf16, tag="o", name="warm")
            for _w in range(20):
                nc.tensor.transpose(
                    warm_ps[:, 0:128], tmpT[:, 0, 0, :], identity16
                )

            # ---------------- main loop over n-blocks ----------------
            pending_out = None  # thunk for previous n-block's out stage

            def emit_out_stage(po, n0):
                # Normalize on ScalarE, transpose back to [c, n] via DMA
                # xbar. Per-q emission so each chain starts as early as
                # possible. Returns a thunk with the blend+store, deferred
                # so the DVE STT never head-blocks the square-path copies.
                otT = outpool.tile([128, KC, NBLK], bf16, tag="otT", name="otT")
                for q in range(4):
                    sq = smallp.tile([128, 1], fp32, name="sq")
                    nc.vector.tensor_scalar(
                        sq,
                        po[q][:, C : C + 1],
                        EPS,
                        1.0 / ALPHA,
                        op0=mybir.AluOpType.max,
                        op1=mybir.AluOpType.mult,
                    )
                    nc.vector.reciprocal(sq, sq)
                    ot = outpool.tile([128, C], bf16, tag="outT", name="outT")
                    nc.scalar.mul(ot, po[q][:, 0:C], sq)
                    for cb in range(KC):
                        nc.sync.dma_start_transpose(
                            otT[:, cb, q * 128 : (q + 1) * 128],
                            ot[:, cb * 128 : (cb + 1) * 128],
                        )

                def blend_and_store():
                    for cb in range(KC):
                        ob = outpool.tile([128, NBLK], fp32, tag="ob", name="ob")
                        nc.vector.scalar_tensor_tensor(
                            ob,
                            sb_dst[:, cb, n0 : n0 + NBLK],
                            ALPHA,
                            otT[:, cb, :],
                            op0=mybir.AluOpType.mult,
                            op1=mybir.AluOpType.add,
                        )
                        nc.sync.dma_start(
                            out_d[cb * 128 : (cb + 1) * 128, n0 : n0 + NBLK], ob
                        )

                return blend_and_store

            pending_blend = None  # deferred blend+store of prev out-stage
            for nb in range(N_NBLK):
                n0 = nb * NBLK
                po = [
                    opool.tile([128, 512], fp32, tag="o", name=f"po{q}")
                    for q in range(4)
                ]
                ps_sim = None
                y2 = None
                mm2_q = []  # deferred 2nd-matmul chunks: (afft, g8, j)

                def emit_mm2_chunk():
                    afft_, g8_, j_ = mm2_q.pop(0)
                    mtg = g8_ * 8 + j_
                    for q in range(4):
                        nc.tensor.matmul(
                            po[q][:, 0 : C + 1],
                            afft_[:, j_, q * 128 : (q + 1) * 128],
                            sb_srcT[:, mtg, 0 : C + 1],
                            start=(mtg == 0),
                            stop=(mtg == MT - 1),
                        )

                for mt in range(MT):
                    gi = mt % 2
                    if gi == 0:
                        ps_sim = pspool.tile(
                            [128, 2, NBLK], fp32, tag="g", name="ps_sim"
                        )
                    # mm1: single fp8 DoubleRow matmul, K=256 (2 k-tiles)
                    nc.tensor.matmul(
                        ps_sim[:, gi, :],
                        csrc[:, :, mt * 128 : (mt + 1) * 128],
                        srcr[:, :, n0 : n0 + NBLK],
                        start=True,
                        stop=True,
                        perf_mode=mybir.MatmulPerfMode.DoubleRow,
                    )
                    if mm2_q:
                        emit_mm2_chunk()
                    if mt % 8 == 0:
                        y2 = y2pool.tile([128, 8, NBLK], bf16, name="y2")
                    if gi == 1:
                        # square the 2-tile group -> y2 half (no scaling:
                        # y2 = (P-mu)^2; Exp applies -1/512)
                        base = ((mt % 8) // 2) * 2
                        g2 = (nb * MT + mt) // 2 % 16
                        if g2 < dve_sq:
                            # DVE may read PSUM on only one input: copy to
                            # SBUF bf16 (1 PSUM read), then square at 2x.
                            zt = zpool.tile([128, 2, NBLK], bf16, name="zt")
                            nc.vector.tensor_copy(zt, ps_sim)
                            nc.vector.tensor_mul(
                                y2[:, base : base + 2, :], zt, zt
                            )
                        else:
                            nc.scalar.activation(
                                y2[:, base : base + 2, :],
                                ps_sim,
                                mybir.ActivationFunctionType.Square,
                                scale=1.0,
                            )
                    if mt % 4 == 3:
                        # Exp in FD=2048 halves, emitted as soon as the 4
                        # y2 slots of the half are complete: shorter ScalarE
                        # FIFO blocks (squares release psum slots sooner)
                        # and mm2 chunks arrive 4-at-a-time (smoother PE
                        # feed, halved end-of-block drain).
                        g8 = mt // 8
                        h = (mt % 8) // 4  # 0 or 1
                        if h == 0:
                            afft = affpool.tile(
                                [128, 8, NBLK], bf16, name="afft"
                            )
                        nc.scalar.activation(
                            afft[:, h * 4 : h * 4 + 4, :],
                            y2[:, h * 4 : h * 4 + 4, :],
                            mybir.ActivationFunctionType.Exp,
                            scale=-SCL2,
                        )
                        for j in range(h * 4, h * 4 + 4):
                            mm2_q.append((afft, g8, j))
                    if mt == 3 and pending_out is not None:
                        pending_blend = pending_out()
                        pending_out = None
                    if mt == 8 and nb == 0:
                        # srcr tail cast, safely mid-stream on the DVE
                        nc.vector.tensor_copy(
                            srcr[:, :, NBLK:], sb_src[:, :, NBLK:HALF]
                        )
                    if mt == 16 and pending_blend is not None:
                        pending_blend()
                        pending_blend = None
                while mm2_q:
                    emit_mm2_chunk()
                pending_out = (lambda po=po, n0=n0: emit_out_stage(po, n0))
            pending_blend = pending_out()
            pending_out = None
            pending_blend()
            pending_blend = None

    nc.compile()
    return nc


def _get_compiled():
    with _LOCK:
        key = (
            os.environ.get("KERNEL_REPS", "1"),
            os.environ.get("KERNEL_DVE_SQ", "11"),
        )
        if key not in _KERNEL_CACHE:
            _KERNEL_CACHE[key] = _build()
        return _KERNEL_CACHE[key]


def _make_in_maps(feature_src, feature_dst):
    src = np.ascontiguousarray(
        np.asarray(feature_src, dtype=np.float32).reshape(B, C, N)
    )
    dst = np.ascontiguousarray(
        np.asarray(feature_dst, dtype=np.float32).reshape(B, C, N)
    )
    in_maps = []
    for core in range(8):
        b, h = core // 2, core % 2
        sl = slice(h * HALF, (h + 1) * HALF)
        in_maps.append(
            {
                # roll so this core's column-half sits at columns 0:HALF;
                # the m-axis permutation cancels in both matmul contractions
                "src": np.ascontiguousarray(np.roll(src[b], -h * HALF, axis=1)),
                "dst": np.ascontiguousarray(dst[b][:, sl]),
            }
        )
    return in_maps


def _assemble(results):
    out = np.empty((B, C, N), dtype=np.float32)
    for core in range(8):
        b, h = core // 2, core % 2
        out[b][:, h * HALF : (h + 1) * HALF] = results[core]["out"]
    return out.reshape(B, C, H, W)


def run(feature_src, feature_dst, trace=False):
    """Run on 8 NeuronCores; returns (output [B,C,H,W], exec_time_ns|None)."""
    from concourse import bass_utils

    nc = _get_compiled()
    in_maps = _make_in_maps(feature_src, feature_dst)
    res = bass_utils.run_bass_kernel_spmd(
        nc, in_maps, core_ids=list(range(8)), trace=trace
    )
    return _assemble(res.results), res.exec_time_ns


def kernel(feature_src, feature_dst):
    out, _ = run(feature_src, feature_dst, trace=False)
    return out


# revision 44
# speedup vs baseline: 1.2967x; 1.0067x over previous
"""Trainium2 Bass kernel for nn_DiffusionModule (self-similarity diffusion).

Math (per batch b, with src = feature_src[b].reshape(C, N)):
    P   = src^T @ src                      # [N, N], sim = P / sqrt(C)
    mu_n = mean_m P[m, n]  (P symmetric)
    aff[n, m] = exp(-((P[n,m] - mu_n) / (16*sqrt(2)))^2)   # sigma=1, C=256
    D = aff / rowsum(aff)
    out = 0.5 * (src @ D^T) + 0.5 * dst

Key identity: P[m,n] - mu_n = sum_c (src[c,m] - sbar[c]) * src[c,n] where
sbar[c] = mean_m src[c,m].  So centering the m-side operand of the first
matmul performs the row-mean subtraction for free (no rank-1 updates).

Sharding: 8 cores = 4 batches x 2 column-halves. SPMD.

Per-core layout (everything in "transposed" [m partitions, n free] space):
  - mm1: simT psum [128m, 512n] = ONE fp8 DoubleRow matmul (K=256 via k-pair)
    using csrc (centered src, fp8) x srcr (raw src cols, fp8)
  - Square -> y2 fp32 (split between ScalarE and VectorE to balance engines)
  - Exp on ScalarE (scale=-1/512 folds the (16*sqrt2))^2 scaling) -> aff bf16
  - mm2: aff chunks as lhsT (K=m), srcT bf16 (with ones column -> row-sums
    land in column 256) as rhs -> out2 psum [128n, 257]
  - normalize rows on DVE, bf16 PE transpose back to [c, n], blend 0.5*dst,
    DMA out.
  - srcT built by DMA-xbar transpose of a bf16 copy of src (no PE/fp32 work).
"""

import os
import threading

import numpy as np

_KERNEL_CACHE = {}
_LOCK = threading.Lock()

B, C, H, W = 4, 256, 64, 64
N = 4096  # H*W
HALF = N // 2  # columns per core
NBLK = 512  # n-block width
N_NBLK = HALF // NBLK  # 4
MT = N // 128  # 32 m-tiles
KC = C // 128  # 2 contraction chunks
SCL2 = 1.0 / 512.0  # ((P-mu)/(16*sqrt2))^2 == SCL2 * (P-mu)^2
ALPHA = 0.5
EPS = 1e-12


def _build():
    """Build + compile the SPMD Bass program once. Returns nc."""
    from contextlib import ExitStack

    import concourse.bass as bass
    import concourse.tile as tile
    from concourse import bacc, mybir
    from concourse.masks import make_identity

    fp32 = mybir.dt.float32
    bf16 = mybir.dt.bfloat16
    fp8 = mybir.dt.float8e4

    nc = bacc.Bacc(
        "TRN2", target_bir_lowering=False, debug=False, num_devices=8
    )

    src_d = nc.dram_tensor("src", [C, N], fp32, kind="ExternalInput").ap()
    dst_d = nc.dram_tensor("dst", [C, HALF], fp32, kind="ExternalInput").ap()
    out_d = nc.dram_tensor("out", [C, HALF], fp32, kind="ExternalOutput").ap()

    reps = int(os.environ.get("KERNEL_REPS", "1"))
    # of every 16 2-tile square groups, this many go to DVE (rest ScalarE)
    dve_sq = int(os.environ.get("KERNEL_DVE_SQ", "11"))


    with tile.TileContext(nc) as tc, ExitStack() as ctx:
        singles = ctx.enter_context(tc.tile_pool(name="singles", bufs=1))
        pspool = ctx.enter_context(tc.tile_pool(name="ps", bufs=2, space="PSUM"))
        opool = ctx.enter_context(tc.tile_pool(name="o", bufs=4, space="PSUM"))
        y2pool = ctx.enter_context(tc.tile_pool(name="y2", bufs=2))
        zpool = ctx.enter_context(tc.tile_pool(name="z", bufs=3))
        affpool = ctx.enter_context(tc.tile_pool(name="aff", bufs=2))
        outpool = ctx.enter_context(tc.tile_pool(name="outsb", bufs=4))
        smallp = ctx.enter_context(tc.tile_pool(name="small", bufs=8))

        for _rep in range(reps):
            # ---------------- stage 0: loads + prep ----------------
            # src DMA'd in 4 half-chunks so the bf16-copy+row-sum (ScalarE
            # Copy + accum_out) starts as each half lands, shortening the
            # serial prep tail before the first mm1.
            sb_src = singles.tile([128, KC, N], fp32)
            srcv = src_d.rearrange("(k p) n -> k p n", p=128)
            for k in range(KC):
                for hh in range(2):
                    nc.sync.dma_start(
                        sb_src[:, k, hh * HALF : (hh + 1) * HALF],
                        srcv[k][:, hh * HALF : (hh + 1) * HALF],
                    )

            identity16 = singles.tile([128, 128], bf16)
            make_identity(nc, identity16)

            src16 = singles.tile([128, KC, N], bf16)
            acc2 = singles.tile([128, KC, 2], fp32)
            negmean = singles.tile([128, KC], fp32)
            csrc = singles.tile([128, KC, N], fp8)
            for k in range(KC):
                for hh in range(2):
                    nc.scalar.activation(
                        src16[:, k, hh * HALF : (hh + 1) * HALF],
                        sb_src[:, k, hh * HALF : (hh + 1) * HALF],
                        mybir.ActivationFunctionType.Copy,
                        accum_out=acc2[:, k, hh : hh + 1],
                    )
                nc.vector.tensor_add(
                    negmean[:, k : k + 1], acc2[:, k, 0:1], acc2[:, k, 1:2]
                )
                nc.vector.tensor_scalar_mul(
                    negmean[:, k : k + 1], negmean[:, k : k + 1], -1.0 / float(N)
                )
                # centered src (fp8) for mm1 lhsT
                nc.vector.tensor_scalar_add(
                    csrc[:, k, :], sb_src[:, k, :], negmean[:, k : k + 1]
                )
            # dst load is only needed at blend time — keep it off the
            # mm1-critical DMA path
            sb_dst = singles.tile([128, KC, HALF], fp32)
            nc.sync.dma_start(sb_dst, dst_d.rearrange("(k p) n -> p k n", p=128))
            # srcr: nb0's slice now; the tail is emitted inside nb0 so it
            # doesn't head-block the DVE queue at main-loop start
            srcr = singles.tile([128, KC, HALF], fp8)
            nc.vector.tensor_copy(srcr[:, :, 0:NBLK], sb_src[:, :, 0:NBLK])

            # srcT [m, c] bf16 with ones column at c=256 (row-sum trick),
            # built by DMA xbar transpose of the bf16 copy of src.
            # The xbar transpose needs a CONTIGUOUS per-partition dst on HW
            # (strided dst produces wrong output), so transpose into tmpT
            # and fan out into the strided srcT layout on DVE.
            # row padded to C+2 so per-mt row stride stays 4B-aligned; the
            # full-tile memset(1.0) supplies the ones column (col 256), the
            # transposed copies overwrite cols 0..255.
            sb_srcT = singles.tile([128, MT, C + 2], bf16)
            nc.gpsimd.memset(sb_srcT, 1.0)
            tmpT = singles.tile([128, KC, MT, 128], bf16)
            for k in range(KC):
                nc.sync.dma_start_transpose(
                    tmpT[:, k],
                    src16[:, k, :],
                )
                nc.vector.tensor_copy(
                    sb_srcT[:, :, k * 128 : (k + 1) * 128], tmpT[:, k]
                )

            # PE warmup: ~4us of transpose spins gated on late-prep data
            # (tmpT k=0) so they run right before the mm1 burst and flip
            # HAM to K=8/8 as the burst begins.
            warm_ps = opool.tile([128, 512], bf16, tag="o", name="warm")
            for _w in range(20):
                nc.tensor.transpose(
                    warm_ps[:, 0:128], tmpT[:, 0, 0, :], identity16
                )

            # ---------------- main loop over n-blocks ----------------
            pending_out = None  # thunk for previous n-block's out stage

            def emit_out_stage(po, n0):
                # Normalize on ScalarE, transpose back to [c, n] via DMA
                # xbar. Per-q emission so each chain starts as early as
                # possible. Returns a thunk with the blend+store, deferred
                # so the DVE STT never head-blocks the square-path copies.
                otT = outpool.tile([128, KC, NBLK], bf16, tag="otT", name="otT")
                for q in range(4):
                    sq = smallp.tile([128, 1], fp32, name="sq")
                    nc.vector.tensor_scalar(
                        sq,
                        po[q][:, C : C + 1],
                        EPS,
                        1.0 / ALPHA,
                        op0=mybir.AluOpType.max,
                        op1=mybir.AluOpType.mult,
                    )
                    nc.vector.reciprocal(sq, sq)
                    ot = outpool.tile([128, C], bf16, tag="outT", name="outT")
                    nc.scalar.mul(ot, po[q][:, 0:C], sq)
                    for cb in range(KC):
                        nc.sync.dma_start_transpose(
                            otT[:, cb, q * 128 : (q + 1) * 128],
                            ot[:, cb * 128 : (cb + 1) * 128],
                        )

                def blend_and_store():
                    for cb in range(KC):
                        ob = outpool.tile([128, NBLK], fp32, tag="ob", name="ob")
                        nc.vector.scalar_tensor_tensor(
                            ob,
                            sb_dst[:, cb, n0 : n0 + NBLK],
                            ALPHA,
                            otT[:, cb, :],
                            op0=mybir.AluOpType.mult,
                            op1=mybir.AluOpType.add,
                        )
                        nc.sync.dma_start(
                            out_d[cb * 128 : (cb + 1) * 128, n0 : n0 + NBLK], ob
                        )

                return blend_and_store

            pending_blend = None  # deferred blend+store of prev out-stage
            for nb in range(N_NBLK):
                n0 = nb * NBLK
                po = [
                    opool.tile([128, 512], fp32, tag="o", name=f"po{q}")
                    for q in range(4)
                ]
                ps_sim = None
                y2 = None
                mm2_q = []  # deferred 2nd-matmul chunks: (afft, g8, j)

                def emit_mm2_chunk():
                    afft_, g8_, j_ = mm2_q.pop(0)
                    mtg = g8_ * 8 + j_
                    for q in range(4):
                        nc.tensor.matmul(
                            po[q][:, 0 : C + 1],
                            afft_[:, j_, q * 128 : (q + 1) * 128],
                            sb_srcT[:, mtg, 0 : C + 1],
                            start=(mtg == 0),
                            stop=(mtg == MT - 1),
                        )

                for mt in range(MT):
                    gi = mt % 2
                    if gi == 0:
                        ps_sim = pspool.tile(
                            [128, 2, NBLK], fp32, tag="g", name="ps_sim"
                        )
                    # mm1: single fp8 DoubleRow matmul, K=256 (2 k-tiles)
                    nc.tensor.matmul(
                        ps_sim[:, gi, :],
                        csrc[:, :, mt * 128 : (mt + 1) * 128],
                        srcr[:, :, n0 : n0 + NBLK],
                        start=True,
                        stop=True,
                        perf_mode=mybir.MatmulPerfMode.DoubleRow,
                    )
                    if mm2_q:
                        emit_mm2_chunk()
                    if mt % 8 == 0:
                        y2 = y2pool.tile([128, 8, NBLK], bf16, name="y2")
                    if gi == 1:
                        # square the 2-tile group -> y2 half (no scaling:
                        # y2 = (P-mu)^2; Exp applies -1/512)
                        base = ((mt % 8) // 2) * 2
                        g2 = (nb * MT + mt) // 2 % 16
                        if g2 < dve_sq:
                            # DVE may read PSUM on only one input: copy to
                            # SBUF bf16 (1 PSUM read), then square at 2x.
                            zt = zpool.tile([128, 2, NBLK], bf16, name="zt")
                            nc.vector.tensor_copy(zt, ps_sim)
                            nc.vector.tensor_mul(
                                y2[:, base : base + 2, :], zt, zt
                            )
                        else:
                            nc.scalar.activation(
                                y2[:, base : base + 2, :],
                                ps_sim,
                                mybir.ActivationFunctionType.Square,
                                scale=1.0,
                            )
                    if mt % 4 == 3:
                        # Exp in FD=2048 halves, emitted as soon as the 4
                        # y2 slots of the half are complete: shorter ScalarE
                        # FIFO blocks (squares release psum slots sooner)
                        # and mm2 chunks arrive 4-at-a-time (smoother PE
                        # feed, halved end-of-block drain).
                        g8 = mt // 8
                        h = (mt % 8) // 4  # 0 or 1
                        if h == 0:
                            afft = affpool.tile(
                                [128, 8, NBLK], bf16, name="afft"
                            )
                        nc.scalar.activation(
                            afft[:, h * 4 : h * 4 + 4, :],
                            y2[:, h * 4 : h * 4 + 4, :],
                            mybir.ActivationFunctionType.Exp,
                            scale=-SCL2,
                        )
                        for j in range(h * 4, h * 4 + 4):
                            mm2_q.append((afft, g8, j))
                    if mt == 3 and pending_out is not None:
                        pending_blend = pending_out()
                        pending_out = None
                    if mt == 8 and nb == 0:
                        # srcr tail cast, safely mid-stream on the DVE
                        nc.vector.tensor_copy(
                            srcr[:, :, NBLK:], sb_src[:, :, NBLK:HALF]
                        )
                    if mt == 16 and pending_blend is not None:
                        pending_blend()
                        pending_blend = None
                while mm2_q:
                    emit_mm2_chunk()
                pending_out = (lambda po=po, n0=n0: emit_out_stage(po, n0))
            pending_blend = pending_out()
            pending_out = None
            pending_blend()
            pending_blend = None

    nc.compile()
    return nc


def _get_compiled():
    with _LOCK:
        key = (
            os.environ.get("KERNEL_REPS", "1"),
            os.environ.get("KERNEL_DVE_SQ", "11"),
        )
        if key not in _KERNEL_CACHE:
            _KERNEL_CACHE[key] = _build()
        return _KERNEL_CACHE[key]


def _make_in_maps(feature_src, feature_dst):
    src = np.ascontiguousarray(
        np.asarray(feature_src, dtype=np.float32).reshape(B, C, N)
    )
    dst = np.ascontiguousarray(
        np.asarray(feature_dst, dtype=np.float32).reshape(B, C, N)
    )
    in_maps = []
    for core in range(8):
        b, h = core // 2, core % 2
        sl = slice(h * HALF, (h + 1) * HALF)
        in_maps.append(
            {
                # roll so this core's column-half sits at columns 0:HALF;
                # the m-axis permutation cancels in both matmul contractions
                "src": np.ascontiguousarray(np.roll(src[b], -h * HALF, axis=1)),
                "dst": np.ascontiguousarray(dst[b][:, sl]),
            }
        )
    return in_maps


def _assemble(results):
    out = np.empty((B, C, N), dtype=np.float32)
    for core in range(8):
        b, h = core // 2, core % 2
        out[b][:, h * HALF : (h + 1) * HALF] = results[core]["out"]
    return out.reshape(B, C, H, W)


def run(feature_src, feature_dst, trace=False):
    """Run on 8 NeuronCores; returns (output [B,C,H,W], exec_time_ns|None)."""
    from concourse import bass_utils

    nc = _get_compiled()
    in_maps = _make_in_maps(feature_src, feature_dst)
    res = bass_utils.run_bass_kernel_spmd(
        nc, in_maps, core_ids=list(range(8)), trace=trace
    )
    return _assemble(res.results), res.exec_time_ns


def kernel(feature_src, feature_dst):
    out, _ = run(feature_src, feature_dst, trace=False)
    return out
